# revision 15
# baseline (speedup 1.0000x reference)
"""Sequence-parallel causal attention for Trainium2, 8 NeuronCores (SPMD).

Problem: o = softmax(causal((q@w_q)(k@w_k)^T/sqrt(dk))) @ (v@w_v) @ w_o
Shapes: q/k/v [8192,1024] f32, w_q/w_k/w_v [1024,128], w_o [128,1024].

v2 design (single-product f32r):
- PE float32r matmuls run at ~1 cycle/row for N>=256 with ~1.5e-4 relative
  precision (HW-measured) -- enough for the 2e-2 gate (calibrated final rel
  err ~3e-3). All hi/lo fp16 splitting is gone.
- Host supplies q/k/v pre-transposed ([8,128,S/8] D-major tiles); no on-device
  transposes or fp16 splits. Weights pre-arranged [128, 8*128].
- Keys sharded: core c projects keys [1024c, 1024c+1024) -> kp^T (f32r) and
  vp (fp16), AllGathered (k first, then v, both overlapped with q/v proj).
- Rows: 8 octave blocks per core (gb = 8*oct + (c or 7-c)) as in v1, but the
  main pass processes 4 octaves at once (rows N up to 512 per matmul).
- Main pass per 128-key tile: 1 f32r QK matmul + 1 fp16 rank-1 (-m) matmul
  + exp (ACT) + 1 fp16 AV matmul + tiny N=1 rowsum matmuls.
- Stats (row max) pass per octave in s-layout + DVE reduce_max.
"""

import math
import numpy as np

N_CORES = 8
S, D, DK = 8192, 1024, 128
RPC = S // N_CORES          # rows per core (1024)
KPC = S // N_CORES          # keys per core (1024)
NOCT = 8                    # octaves (128-row blocks per core)
NEG_BIG = -2.0e9
INV_SQRT_DK = 1.0 / math.sqrt(DK)

_CACHE = {}


def _build():
    import concourse.bass as bass
    import concourse.mybir as mybir
    import concourse.tile as tile
    from concourse import bacc
    from contextlib import ExitStack

    dt = mybir.dt
    f32, f16, f32r = dt.float32, dt.float16, dt.float32r

    nc = bacc.Bacc("TRN2", target_bir_lowering=False, debug=False,
                   num_devices=N_CORES)

    # ---- I/O (host supplies transposed, pre-arranged tensors) ----
    qT_p = nc.declare_dram_parameter("qT", [8, 128, RPC], f32r, isOutput=False)
    kT_p = nc.declare_dram_parameter("kT", [8, 128, KPC], f32r, isOutput=False)
    vT_p = nc.declare_dram_parameter("vT", [8, 128, KPC], f16, isOutput=False)
    wq_p = nc.declare_dram_parameter("wq", [128, 8 * DK], f32r, isOutput=False)
    wk_p = nc.declare_dram_parameter("wk", [128, 8 * DK], f32r, isOutput=False)
    wv_p = nc.declare_dram_parameter("wv", [128, 8 * DK], f16, isOutput=False)
    wo_p = nc.declare_dram_parameter("wo", [DK, D], f32r, isOutput=False)
    maskT_p = nc.declare_dram_parameter("maskT", [2, 128, 1024], f32, isOutput=False)
    mask2_p = nc.declare_dram_parameter("mask2", [2, 128, 1024], f32, isOutput=False)
    ident_p = nc.declare_dram_parameter("ident", [128, 128], f32, isOutput=False)
    nones_p = nc.declare_dram_parameter("negones", [1, 128], f16, isOutput=False)
    ones_p = nc.declare_dram_parameter("ones_col", [128, 1], f16, isOutput=False)
    o_sh = nc.declare_dram_parameter("o_sh", [RPC, D], f32, isOutput=True)

    # ---- internal DRAM for collectives ----
    agk_in = nc.dram_tensor("agk_in", [128, KPC], f32r)
    agk_out = nc.dram_tensor("agk_out", [N_CORES, 128, KPC], f32r,
                             addr_space="Shared")
    agv_in = nc.dram_tensor("agv_in", [KPC, DK], f16)
    agv_out = nc.dram_tensor("agv_out", [N_CORES, KPC, DK], f16,
                             addr_space="Shared")

    rgroups = [list(range(N_CORES))]

    with tile.TileContext(nc) as tc, ExitStack() as ctx:
        consts = ctx.enter_context(tc.tile_pool(name="consts", bufs=1))
        persist = ctx.enter_context(tc.tile_pool(name="persist", bufs=1))

        # constants / weights
        wq_sb = consts.tile([128, 8 * DK], f32r, tag="wq")
        nc.sync.dma_start(wq_sb[:], wq_p[:])
        wk_sb = consts.tile([128, 8 * DK], f32r, tag="wk")
        nc.sync.dma_start(wk_sb[:], wk_p[:])
        wv_sb = consts.tile([128, 8 * DK], f16, tag="wv")
        nc.sync.dma_start(wv_sb[:], wv_p[:])
        wo_sb = consts.tile([128, D], f32r, tag="wo")
        nc.sync.dma_start(wo_sb[:], wo_p[:])
        ident = consts.tile([128, 128], f32, tag="ident")
        nc.sync.dma_start(ident[:], ident_p[:])
        negones = consts.tile([1, 128], f16, tag="negones")
        nc.sync.dma_start(negones[:], nones_p[:])
        ones_col = consts.tile([128, 1], f16, tag="ones")
        nc.sync.dma_start(ones_col[:], ones_p[:])
        maskT_sb = [consts.tile([128, 1024], f32, tag=f"maskT{p}", name=f"maskT{p}")
                    for p in range(2)]
        mask2_sb = [consts.tile([128, 1024], f32, tag=f"mask2{p}", name=f"mask2{p}")
                    for p in range(2)]
        for p in range(2):
            nc.sync.dma_start(maskT_sb[p][:], maskT_p[p])
            nc.sync.dma_start(mask2_sb[p][:], mask2_p[p])

        # persistent activations
        qpT = persist.tile([128, RPC], f32r, tag="qpT")
        kpT = persist.tile([128, S], f32r, tag="kpT")
        vp_sb = [persist.tile([128, 8 * DK], f16, tag=f"vp{g}", name=f"vp{g}")
                 for g in range(8)]

        # ---------- phase A: projections + gathers ----------
        with tc.tile_pool(name="xin", bufs=9) as xin, \
             tc.tile_pool(name="projps", bufs=2, space="PSUM") as projps, \
             tc.tile_pool(name="projsb", bufs=4) as projsb:

            # ---- k ----
            kT_in = [xin.tile([128, KPC], f32r, tag="xin", name="kT_in")
                     for _ in range(8)]
            for di in range(8):
                nc.sync.dma_start(kT_in[di][:], kT_p[di])
            kp_loc = projsb.tile([128, KPC], f32r, tag="kploc", name="kploc")
            for half in range(2):
                ps = projps.tile([128, 512], f32, tag="projps")
                sl = slice(512 * half, 512 * (half + 1))
                for di in range(8):
                    nc.tensor.matmul(ps[:], wk_sb[:, 128 * di:128 * (di + 1)],
                                     kT_in[di][:, sl], start=(di == 0),
                                     stop=(di == 7))
                nc.scalar.copy(kp_loc[:, sl], ps[:])
            nc.sync.dma_start(agk_in[:], kp_loc[:])
            nc.gpsimd.collective_compute(
                "AllGather", mybir.AluOpType.bypass, replica_groups=rgroups,
                ins=[agk_in[:]], outs=[agk_out[:]])

            # ---- v (fp16): out [keys, dv] per 128-key block ----
            vT_in = [xin.tile([128, KPC], f16, tag="xin", name="vT_in")
                     for _ in range(8)]
            for di in range(8):
                nc.sync.dma_start(vT_in[di][:], vT_p[di])
            for kt in range(8):
                ps = projps.tile([128, 128], f32, tag="projpsv")
                ksl = slice(128 * kt, 128 * (kt + 1))
                for di in range(8):
                    nc.tensor.matmul(ps[:], vT_in[di][:, ksl],
                                     wv_sb[:, 128 * di:128 * (di + 1)],
                                     start=(di == 0), stop=(di == 7))
                vh16 = projsb.tile([128, 128], f16, tag="vh16")
                nc.scalar.copy(vh16[:], ps[:])
                nc.sync.dma_start(agv_in[ksl, :], vh16[:])
            nc.gpsimd.collective_compute(
                "AllGather", mybir.AluOpType.bypass, replica_groups=rgroups,
                ins=[agv_in[:]], outs=[agv_out[:]])

            # ---- q ----
            qT_in = [xin.tile([128, RPC], f32r, tag="xin", name="qT_in")
                     for _ in range(8)]
            for di in range(8):
                nc.sync.dma_start(qT_in[di][:], qT_p[di])
            for half in range(2):
                ps = projps.tile([128, 512], f32, tag="projps")
                sl = slice(512 * half, 512 * (half + 1))
                for di in range(8):
                    nc.tensor.matmul(ps[:], wq_sb[:, 128 * di:128 * (di + 1)],
                                     qT_in[di][:, sl], start=(di == 0),
                                     stop=(di == 7))
                nc.scalar.copy(qpT[:, sl], ps[:])

            # ---- gathered loads (split per source for earlier stats start) ----
            for g in range(8):
                nc.sync.dma_start(kpT[:, KPC * g:KPC * (g + 1)], agk_out[g])
            for g in range(8):
                nc.sync.dma_start(
                    vp_sb[g][:].rearrange("p (rb dv) -> p rb dv", rb=8),
                    agv_out[g].rearrange("a b -> (a b)")
                    .rearrange("(rb p dv) -> p rb dv", rb=8, p=128))

        # ---------- phase C: attention, two row-groups of 4 octaves ----------
        with tc.tile_pool(name="statps", bufs=2, space="PSUM") as statps, \
             tc.tile_pool(name="mainps", bufs=2, space="PSUM") as mainps, \
             tc.tile_pool(name="oTps", bufs=1, space="PSUM") as oTps, \
             tc.tile_pool(name="miscps", bufs=1, space="PSUM") as miscps, \
             tc.tile_pool(name="mpool", bufs=3) as mpool, \
             tc.tile_pool(name="rsump", bufs=2) as rsump, \
             tc.tile_pool(name="pTpool", bufs=3) as pTpool, \
             tc.tile_pool(name="accp", bufs=2) as accp, \
             tc.tile_pool(name="osb", bufs=4) as osb:

            # row max per octave, one column per oct (unscaled logits)
            m_all = persist.tile([128, 8], f32, tag="m_all")
            mrep_g = [persist.tile([1, 512], f16, tag=f"mrep{g}", name=f"mrep{g}")
                      for g in range(2)]
            mxs_t = {}

            def emit_stat_chunk(oct_, st):
                par = oct_ % 2
                ngrp = 2 * (oct_ + 1)
                if st == 0:
                    mxs_t[oct_] = mpool.tile([128, ngrp], f32, tag="mxs",
                                             name=f"mxs{oct_}")
                mxs = mxs_t[oct_]
                ps_s = statps.tile([128, 512], f32, tag="stat")
                nc.tensor.matmul(ps_s[:], qpT[:, 128 * oct_:128 * (oct_ + 1)],
                                 kpT[:, 512 * st:512 * (st + 1)],
                                 start=True, stop=True)
                if st >= 2 * oct_:
                    w = st - 2 * oct_
                    nc.vector.tensor_add(
                        ps_s[:], ps_s[:],
                        mask2_sb[par][:, 512 * w:512 * (w + 1)])
                nc.vector.reduce_max(mxs[:, st:st + 1], ps_s[:],
                                     axis=mybir.AxisListType.X)
                if st == ngrp - 1:
                    nc.vector.reduce_max(m_all[:, oct_:oct_ + 1], mxs[:],
                                         axis=mybir.AxisListType.X)

            def emit_mrep(g):
                for s in range(4):
                    mps = miscps.tile([1, 128], f32, tag="mrepps")
                    nc.tensor.transpose(mps[:], m_all[:, 4 * g + s:4 * g + s + 1],
                                        ident[:])
                    nc.scalar.mul(mrep_g[g][:, 128 * s:128 * (s + 1)], mps[:],
                                  1.0 / 16.0)

            def emit_main_group(g, pending):
                n_t = 32 if g == 0 else 64
                rbase = 512 * g
                oT = oTps.tile([128, 512], f32, tag="oT", name=f"oT{g}")
                acc = accp.tile([128, 512], f16, tag="acc", name=f"acc{g}")
                nc.gpsimd.memset(acc[:], 0.0)
                pTs = {}

                def emit_front(j):
                    # QK + (-m) rank-1 into PSUM, mask, exp -> pT (fp16)
                    i = max(0, j // 8 - 4 * g)
                    rsl = slice(rbase + 128 * i, rbase + 512)
                    psl = slice(128 * i, 512)
                    ps_g = mainps.tile([128, 512], f32, tag="sT")
                    nc.tensor.matmul(ps_g[:, psl],
                                     kpT[:, 128 * j:128 * (j + 1)],
                                     qpT[:, rsl], start=True, stop=False)
                    nc.tensor.matmul(ps_g[:, psl], negones[:],
                                     mrep_g[g][:, psl], start=False, stop=True)
                    d = j // 8 - 4 * g
                    if d >= 0:
                        w = j % 8
                        nc.vector.tensor_add(
                            ps_g[:, 128 * i:128 * (i + 1)],
                            ps_g[:, 128 * i:128 * (i + 1)],
                            maskT_sb[(4 * g + d) % 2][:, 128 * w:128 * (w + 1)])
                    pT = pTpool.tile([128, 512], f16, tag="pT")
                    nc.scalar.activation(pT[:, psl], ps_g[:, psl],
                                         mybir.ActivationFunctionType.Exp,
                                         scale=INV_SQRT_DK)
                    pTs[j] = pT

                def emit_back(j):
                    # AV accumulate + Pool rowsum add + octave epilogue
                    i = max(0, j // 8 - 4 * g)
                    psl = slice(128 * i, 512)
                    pT = pTs.pop(j)
                    g2, rb = j // 8, j % 8
                    vtile = vp_sb[g2][:, 128 * rb:128 * (rb + 1)]
                    d = j // 8 - 4 * g
                    boundary = (d >= 0 and j % 8 == 7)
                    if boundary:
                        nc.tensor.matmul(oT[:, 128 * i:128 * (i + 1)], vtile,
                                         pT[:, 128 * i:128 * (i + 1)],
                                         start=(j == 0), stop=True)
                        if i < 3:
                            nc.tensor.matmul(oT[:, 128 * (i + 1):512], vtile,
                                             pT[:, 128 * (i + 1):512],
                                             start=(j == 0), stop=False)
                    else:
                        nc.tensor.matmul(oT[:, psl], vtile, pT[:, psl],
                                         start=(j == 0), stop=False)
                    nc.gpsimd.tensor_add(acc[:, psl], acc[:, psl], pT[:, psl])
                    if boundary:
                        s = i
                        oct_ = 4 * g + s
                        ssl = slice(128 * s, 128 * (s + 1))
                        oT_sb = osb.tile([128, 128], f32r, tag="oTsb")
                        nc.scalar.copy(oT_sb[:], oT[:, ssl])
                        ps_sm = miscps.tile([128, 1], f32, tag="smr")
                        nc.tensor.matmul(ps_sm[:], acc[:, ssl], ones_col[:],
                                         start=True, stop=True)
                        rsum = rsump.tile([128, 1], f32, tag="rsum")
                        nc.vector.reciprocal(rsum[:], ps_sm[:])
                        out_full = osb.tile([128, 1024], f32, tag="outfull",
                                            name="outfull")
                        for half in range(2):
                            osl = slice(512 * half, 512 * (half + 1))
                            ps_o = miscps.tile([128, 512], f32, tag="pso")
                            nc.tensor.matmul(ps_o[:], oT_sb[:], wo_sb[:, osl],
                                             start=True, stop=True)
                            nc.scalar.activation(
                                out_full[:, osl], ps_o[:],
                                mybir.ActivationFunctionType.Copy,
                                scale=rsum[:])
                        nc.sync.dma_start(o_sh[128 * oct_:128 * (oct_ + 1), :],
                                          out_full[:])

                for j in range(n_t + 1):
                    if j < n_t:
                        emit_front(j)
                    if j > 0:
                        emit_back(j - 1)
                    # interleave pending group-1 stats work
                    for _ in range(2):
                        if pending:
                            oct2, st2 = pending.pop(0)
                            emit_stat_chunk(oct2, st2)

            # group-0 stats, then main g0 with g1 stats interleaved
            for oct_ in range(4):
                for st in range(2 * (oct_ + 1)):
                    emit_stat_chunk(oct_, st)
            emit_mrep(0)
            pending = [(oct_, st) for oct_ in range(4, 8)
                       for st in range(2 * (oct_ + 1))]
            emit_main_group(0, pending)
            while pending:
                oct2, st2 = pending.pop(0)
                emit_stat_chunk(oct2, st2)
            emit_mrep(1)
            emit_main_group(1, [])

    nc.compile()
    return nc


def _host_inputs(q, k, v, w_q, w_k, w_v, w_o):
    """Build per-core input maps (host-side sharding + transposes)."""
    f16 = np.float16

    ident = np.eye(128, dtype=np.float32)
    negones = np.full((1, 128), -16.0, dtype=f16)
    ones_col = np.ones((128, 1), dtype=f16)

    # weights pre-arranged [128, 8*DK]: w_sb[p, di*DK+dk] = w[128*di+p, dk]
    def warr(w, dtype):
        return np.ascontiguousarray(
            w.reshape(8, 128, DK).transpose(1, 0, 2).reshape(128, 8 * DK)
        ).astype(dtype)
    wq_a = warr(w_q, np.float32)
    wk_a = warr(w_k, np.float32)
    wv_a = warr(w_v, f16)

    kidx = np.arange(128)[:, None]
    t_f = np.arange(1024)[None, :] // 128
    r_f = np.arange(1024)[None, :] % 128
    ridx = np.arange(128)[:, None]
    kk_f = np.arange(1024)[None, :]

    in_maps = []
    for c in range(N_CORES):
        blocks = [8 * o + (c if o % 2 == 0 else 7 - c) for o in range(NOCT)]
        q_rows = np.concatenate([q[128 * gb:128 * (gb + 1)] for gb in blocks])
        qT = np.ascontiguousarray(q_rows.T.reshape(8, 128, RPC))
        k_sh = k[KPC * c:KPC * (c + 1)]
        kT = np.ascontiguousarray(k_sh.T.reshape(8, 128, KPC))
        v_sh = v[KPC * c:KPC * (c + 1)]
        vT = np.ascontiguousarray(v_sh.T.reshape(8, 128, KPC)).astype(f16)
        maskT = np.empty((2, 128, 1024), np.float32)
        mask2 = np.empty((2, 128, 1024), np.float32)
        for p, pos in enumerate((c, 7 - c)):
            maskT[p] = np.where(128 * t_f + kidx <= 128 * pos + r_f, 0.0, NEG_BIG)
            mask2[p] = np.where(kk_f <= 128 * pos + ridx, 0.0, NEG_BIG)
        in_maps.append({
            "qT": qT, "kT": kT, "vT": vT,
            "wq": wq_a, "wk": wk_a, "wv": wv_a, "wo": w_o,
            "maskT": maskT, "mask2": mask2, "ident": ident,
            "negones": negones, "ones_col": ones_col,
        })
    return in_maps


def kernel(q, k, v, w_q, w_k, w_v, w_o):
    from concourse.bass_utils import run_bass_kernel_spmd

    q = np.asarray(q, dtype=np.float32)
    k = np.asarray(k, dtype=np.float32)
    v = np.asarray(v, dtype=np.float32)
    w_q = np.asarray(w_q, dtype=np.float32)
    w_k = np.asarray(w_k, dtype=np.float32)
    w_v = np.asarray(w_v, dtype=np.float32)
    w_o = np.asarray(w_o, dtype=np.float32)

    if "nc" not in _CACHE:
        _CACHE["nc"] = _build()
    nc = _CACHE["nc"]

    in_maps = _host_inputs(q, k, v, w_q, w_k, w_v, w_o)
    res = run_bass_kernel_spmd(nc, in_maps, list(range(N_CORES)))

    out = np.empty((S, D), dtype=np.float32)
    for c in range(N_CORES):
        o_sh = res.results[c]["o_sh"]
        for o in range(NOCT):
            gb = 8 * o + (c if o % 2 == 0 else 7 - c)
            out[128 * gb:128 * (gb + 1)] = o_sh[128 * o:128 * (o + 1)]
    return out


# revision 33
# speedup vs baseline: 1.6646x; 1.6646x over previous
"""Sequence-parallel causal attention for Trainium2, 8 NeuronCores (SPMD).

Problem: o = softmax(causal((q@w_q)(k@w_k)^T/sqrt(dk))) @ (v@w_v) @ w_o
Shapes: q/k/v [8192,1024] f32, w_q/w_k/w_v [1024,128], w_o [128,1024].

v2 design (single-product f32r):
- PE float32r matmuls run at ~1 cycle/row for N>=256 with ~1.5e-4 relative
  precision (HW-measured) -- enough for the 2e-2 gate (calibrated final rel
  err ~3e-3). All hi/lo fp16 splitting is gone.
- Host supplies q/k/v pre-transposed ([8,128,S/8] D-major tiles); no on-device
  transposes or fp16 splits. Weights pre-arranged [128, 8*128].
- Keys sharded: core c projects keys [1024c, 1024c+1024) -> kp^T (f32r) and
  vp (fp16), AllGathered (k first, then v, both overlapped with q/v proj).
- Rows: 8 octave blocks per core (gb = 8*oct + (c or 7-c)) as in v1, but the
  main pass processes 4 octaves at once (rows N up to 512 per matmul).
- Main pass per 128-key tile: 1 f32r QK matmul + 1 fp16 rank-1 (-m) matmul
  + exp (ACT) + 1 fp16 AV matmul + tiny N=1 rowsum matmuls.
- Stats (row max) pass per octave in s-layout + DVE reduce_max.
"""

import math
import numpy as np

N_CORES = 8
S, D, DK = 8192, 1024, 128
RPC = S // N_CORES          # rows per core (1024)
KPC = S // N_CORES          # keys per core (1024)
NOCT = 8                    # octaves (128-row blocks per core)
NEG_BIG = -2.0e9
INV_SQRT_DK = 1.0 / math.sqrt(DK)

_CACHE = {}


def _build():
    import concourse.bass as bass
    import concourse.mybir as mybir
    import concourse.tile as tile
    from concourse import bacc
    from contextlib import ExitStack

    dt = mybir.dt
    f32, f16, f32r = dt.float32, dt.float16, dt.float32r

    nc = bacc.Bacc("TRN2", target_bir_lowering=False, debug=False,
                   num_devices=N_CORES)

    # ---- I/O (host supplies transposed, pre-arranged tensors) ----
    qT_p = nc.declare_dram_parameter("qT", [8, 128, RPC], f32r, isOutput=False)
    kT_p = nc.declare_dram_parameter("kT", [8, 128, KPC], f32r, isOutput=False)
    vT_p = nc.declare_dram_parameter("vT", [8, 128, KPC], f16, isOutput=False)
    wq_p = nc.declare_dram_parameter("wq", [128, 8 * DK], f32r, isOutput=False)
    wk_p = nc.declare_dram_parameter("wk", [128, 8 * DK], f32r, isOutput=False)
    wv_p = nc.declare_dram_parameter("wv", [128, 8 * DK], f16, isOutput=False)
    wo_p = nc.declare_dram_parameter("wo", [DK, D], f32r, isOutput=False)
    maskT_p = nc.declare_dram_parameter("maskT", [2, 128, 1024], f32, isOutput=False)
    mask2_p = nc.declare_dram_parameter("mask2", [2, 128, 1024], f32, isOutput=False)
    ident_p = nc.declare_dram_parameter("ident", [128, 128], f32, isOutput=False)
    nones_p = nc.declare_dram_parameter("negones", [1, 128], f16, isOutput=False)
    ones_p = nc.declare_dram_parameter("ones_col", [128, 1], f16, isOutput=False)
    o_sh = nc.declare_dram_parameter("o_sh", [RPC, D], f32, isOutput=True)

    # ---- internal DRAM for collectives ----
    agk_in = nc.dram_tensor("agk_in", [128, KPC], f32r)
    agk_out = nc.dram_tensor("agk_out", [N_CORES, 128, KPC], f32r,
                             addr_space="Shared")
    agv_in = nc.dram_tensor("agv_in", [KPC, DK], f16)
    agv_out = nc.dram_tensor("agv_out", [N_CORES, KPC, DK], f16,
                             addr_space="Shared")

    rgroups = [list(range(N_CORES))]

    with tile.TileContext(nc) as tc, ExitStack() as ctx:
        consts = ctx.enter_context(tc.tile_pool(name="consts", bufs=1))
        persist = ctx.enter_context(tc.tile_pool(name="persist", bufs=1))

        # constants / weights (tiles allocated up front; DMAs ordered so the
        # k-projection critical path -- wk + kT -- goes first)
        wq_sb = consts.tile([128, 8 * DK], f32r, tag="wq")
        wk_sb = consts.tile([128, 8 * DK], f32r, tag="wk")
        wv_sb = consts.tile([128, 8 * DK], f16, tag="wv")
        wo_sb = consts.tile([128, D], f32r, tag="wo")
        ident = consts.tile([128, 128], f32, tag="ident")
        negones = consts.tile([1, 128], f16, tag="negones")
        ones_col = consts.tile([128, 1], f16, tag="ones")
        maskT_sb = [consts.tile([128, 1024], f32, tag=f"maskT{p}", name=f"maskT{p}")
                    for p in range(2)]
        mask2_sb = [consts.tile([128, 1024], f32, tag=f"mask2{p}", name=f"mask2{p}")
                    for p in range(2)]
        nc.sync.dma_start(wk_sb[:], wk_p[:])

        # persistent activations
        qpT = persist.tile([128, RPC], f32r, tag="qpT")
        kpT = persist.tile([128, S], f32r, tag="kpT")
        vp_sb = [persist.tile([128, 8 * DK], f16, tag=f"vp{g}", name=f"vp{g}")
                 for g in range(8)]

        # ---------- phase A: projections + gathers ----------
        with tc.tile_pool(name="xin", bufs=9) as xin, \
             tc.tile_pool(name="projps", bufs=2, space="PSUM") as projps, \
             tc.tile_pool(name="projsb", bufs=4) as projsb:

            # ---- k ----
            kT_in = [xin.tile([128, KPC], f32r, tag="xin", name="kT_in")
                     for _ in range(8)]
            for di in range(8):
                nc.sync.dma_start(kT_in[di][:], kT_p[di])
            kp_loc = projsb.tile([128, KPC], f32r, tag="kploc", name="kploc")
            for half in range(2):
                ps = projps.tile([128, 512], f32, tag="projps")
                sl = slice(512 * half, 512 * (half + 1))
                for di in range(8):
                    nc.tensor.matmul(ps[:], wk_sb[:, 128 * di:128 * (di + 1)],
                                     kT_in[di][:, sl], start=(di == 0),
                                     stop=(di == 7))
                nc.scalar.copy(kp_loc[:, sl], ps[:])
            nc.sync.dma_start(agk_in[:], kp_loc[:])
            nc.gpsimd.collective_compute(
                "AllGather", mybir.AluOpType.bypass, replica_groups=rgroups,
                ins=[agk_in[:]], outs=[agk_out[:]])

            # ---- v (fp16): out [keys, dv] per 128-key block ----
            vT_in = [xin.tile([128, KPC], f16, tag="xin", name="vT_in")
                     for _ in range(8)]
            nc.sync.dma_start(wv_sb[:], wv_p[:])
            for di in range(8):
                nc.sync.dma_start(vT_in[di][:], vT_p[di])
            for kt in range(8):
                ps = projps.tile([128, 128], f32, tag="projpsv")
                ksl = slice(128 * kt, 128 * (kt + 1))
                for di in range(8):
                    nc.tensor.matmul(ps[:], vT_in[di][:, ksl],
                                     wv_sb[:, 128 * di:128 * (di + 1)],
                                     start=(di == 0), stop=(di == 7))
                vh16 = projsb.tile([128, 128], f16, tag="vh16")
                nc.scalar.copy(vh16[:], ps[:])
                nc.sync.dma_start(agv_in[ksl, :], vh16[:])
            nc.gpsimd.collective_compute(
                "AllGather", mybir.AluOpType.bypass, replica_groups=rgroups,
                ins=[agv_in[:]], outs=[agv_out[:]])

            # ---- q ----
            qT_in = [xin.tile([128, RPC], f32r, tag="xin", name="qT_in")
                     for _ in range(8)]
            nc.sync.dma_start(wq_sb[:], wq_p[:])
            for di in range(8):
                nc.sync.dma_start(qT_in[di][:], qT_p[di])
            # remaining constants (needed from stats onward)
            nc.sync.dma_start(wo_sb[:], wo_p[:])
            nc.sync.dma_start(ident[:], ident_p[:])
            nc.sync.dma_start(negones[:], nones_p[:])
            nc.sync.dma_start(ones_col[:], ones_p[:])
            for p in range(2):
                nc.sync.dma_start(maskT_sb[p][:], maskT_p[p])
                nc.sync.dma_start(mask2_sb[p][:], mask2_p[p])
            for half in range(2):
                ps = projps.tile([128, 512], f32, tag="projps")
                sl = slice(512 * half, 512 * (half + 1))
                for di in range(8):
                    nc.tensor.matmul(ps[:], wq_sb[:, 128 * di:128 * (di + 1)],
                                     qT_in[di][:, sl], start=(di == 0),
                                     stop=(di == 7))
                nc.scalar.copy(qpT[:, sl], ps[:])

            # ---- gathered loads (split per source for earlier stats start) ----
            for g in range(8):
                nc.sync.dma_start(kpT[:, KPC * g:KPC * (g + 1)], agk_out[g])
            for g in range(8):
                nc.sync.dma_start(
                    vp_sb[g][:].rearrange("p (rb dv) -> p rb dv", rb=8),
                    agv_out[g].rearrange("a b -> (a b)")
                    .rearrange("(rb p dv) -> p rb dv", rb=8, p=128))

        # ---------- phase C: attention, two row-groups of 4 octaves ----------
        with tc.tile_pool(name="statps", bufs=2, space="PSUM") as statps, \
             tc.tile_pool(name="mainps", bufs=3, space="PSUM") as mainps, \
             tc.tile_pool(name="oTps", bufs=1, space="PSUM") as oTps, \
             tc.tile_pool(name="miscps", bufs=1, space="PSUM") as miscps, \
             tc.tile_pool(name="mpool", bufs=3) as mpool, \
             tc.tile_pool(name="rsump", bufs=2) as rsump, \
             tc.tile_pool(name="pTpool", bufs=34) as pTpool, \
             tc.tile_pool(name="accp", bufs=2) as accp, \
             tc.tile_pool(name="osb", bufs=4) as osb:

            # row max per octave, one column per oct (unscaled logits)
            m_all = persist.tile([128, 8], f32, tag="m_all")
            mrep_g = [persist.tile([1, 512], f16, tag=f"mrep{g}", name=f"mrep{g}")
                      for g in range(2)]
            mxs_t = {}

            def emit_stat_chunk(oct_, st):
                par = oct_ % 2
                ngrp = 2 * (oct_ + 1)
                if st == 0:
                    mxs_t[oct_] = mpool.tile([128, ngrp], f32, tag="mxs",
                                             name=f"mxs{oct_}")
                mxs = mxs_t[oct_]
                ps_s = statps.tile([128, 512], f32, tag="stat")
                nc.tensor.matmul(ps_s[:], qpT[:, 128 * oct_:128 * (oct_ + 1)],
                                 kpT[:, 512 * st:512 * (st + 1)],
                                 start=True, stop=True)
                if st >= 2 * oct_:
                    w = st - 2 * oct_
                    nc.vector.tensor_add(
                        ps_s[:], ps_s[:],
                        mask2_sb[par][:, 512 * w:512 * (w + 1)])
                nc.vector.reduce_max(mxs[:, st:st + 1], ps_s[:],
                                     axis=mybir.AxisListType.X)
                if st == ngrp - 1:
                    nc.vector.reduce_max(m_all[:, oct_:oct_ + 1], mxs[:],
                                         axis=mybir.AxisListType.X)

            def emit_mrep(g):
                for s in range(4):
                    mps = miscps.tile([1, 128], f32, tag="mrepps")
                    nc.tensor.transpose(mps[:], m_all[:, 4 * g + s:4 * g + s + 1],
                                        ident[:])
                    nc.scalar.mul(mrep_g[g][:, 128 * s:128 * (s + 1)], mps[:],
                                  1.0 / 16.0)

            def emit_main_group(g, pending):
                n_t = 32 if g == 0 else 64
                rbase = 512 * g
                oT = oTps.tile([128, 512], f32, tag="oT", name=f"oT{g}")
                acc = accp.tile([128, 512], f16, tag="acc", name=f"acc{g}")
                acc2 = accp.tile([128, 512], f16, tag="acc2", name=f"acc2{g}")
                nc.gpsimd.memset(acc[:], 0.0)
                nc.gpsimd.memset(acc2[:], 0.0)
                pTs = {}

                def emit_front(j):
                    # QK + (-m) rank-1 into PSUM, mask, exp -> pT (fp16)
                    i = max(0, j // 8 - 4 * g)
                    rsl = slice(rbase + 128 * i, rbase + 512)
                    psl = slice(128 * i, 512)
                    ps_g = mainps.tile([128, 512], f32, tag="sT")
                    nc.tensor.matmul(ps_g[:, psl],
                                     kpT[:, 128 * j:128 * (j + 1)],
                                     qpT[:, rsl], start=True, stop=False)
                    nc.tensor.matmul(ps_g[:, psl], negones[:],
                                     mrep_g[g][:, psl], start=False, stop=True)
                    d = j // 8 - 4 * g
                    if d >= 0:
                        w = j % 8
                        nc.vector.tensor_add(
                            ps_g[:, 128 * i:128 * (i + 1)],
                            ps_g[:, 128 * i:128 * (i + 1)],
                            maskT_sb[(4 * g + d) % 2][:, 128 * w:128 * (w + 1)])
                    pT = pTpool.tile([128, 512], f16, tag="pT")
                    nc.scalar.activation(pT[:, psl], ps_g[:, psl],
                                         mybir.ActivationFunctionType.Exp,
                                         scale=INV_SQRT_DK)
                    pTs[j] = pT

                def emit_back(j):
                    # AV accumulate + Pool rowsum add + octave epilogue
                    i = max(0, j // 8 - 4 * g)
                    psl = slice(128 * i, 512)
                    pT = pTs.pop(j)
                    g2, rb = j // 8, j % 8
                    vtile = vp_sb[g2][:, 128 * rb:128 * (rb + 1)]
                    d = j // 8 - 4 * g
                    boundary = (d >= 0 and j % 8 == 7)
                    if boundary:
                        nc.tensor.matmul(oT[:, 128 * i:128 * (i + 1)], vtile,
                                         pT[:, 128 * i:128 * (i + 1)],
                                         start=(j == 0), stop=True)
                        if i < 3:
                            nc.tensor.matmul(oT[:, 128 * (i + 1):512], vtile,
                                             pT[:, 128 * (i + 1):512],
                                             start=(j == 0), stop=False)
                    else:
                        nc.tensor.matmul(oT[:, psl], vtile, pT[:, psl],
                                         start=(j == 0), stop=False)
                    if j % 2 == 0:
                        nc.vector.tensor_add(acc[:, psl], acc[:, psl], pT[:, psl])
                    else:
                        nc.gpsimd.tensor_add(acc2[:, psl], acc2[:, psl], pT[:, psl])
                    if boundary:
                        s = i
                        oct_ = 4 * g + s
                        ssl = slice(128 * s, 128 * (s + 1))
                        oT_sb = osb.tile([128, 128], f32r, tag="oTsb")
                        nc.scalar.copy(oT_sb[:], oT[:, ssl])
                        nc.vector.tensor_add(acc[:, ssl], acc[:, ssl],
                                             acc2[:, ssl])
                        ps_sm = miscps.tile([128, 1], f32, tag="smr")
                        nc.tensor.matmul(ps_sm[:], acc[:, ssl], ones_col[:],
                                         start=True, stop=True)
                        rsum = rsump.tile([128, 1], f32, tag="rsum")
                        nc.vector.reciprocal(rsum[:], ps_sm[:])
                        out_full = osb.tile([128, 1024], f32, tag="outfull",
                                            name="outfull")
                        for half in range(2):
                            osl = slice(512 * half, 512 * (half + 1))
                            ps_o = statps.tile([128, 512], f32, tag="stat")
                            nc.tensor.matmul(ps_o[:], oT_sb[:], wo_sb[:, osl],
                                             start=True, stop=True)
                            nc.scalar.activation(
                                out_full[:, osl], ps_o[:],
                                mybir.ActivationFunctionType.Copy,
                                scale=rsum[:])
                        nc.sync.dma_start(o_sh[128 * oct_:128 * (oct_ + 1), :],
                                          out_full[:])

                if g == 0:
                    # two-phase: all fronts (+ all interleaved g1 stats),
                    # then all backs -- only the AV tail waits on AG(v)
                    for j in range(n_t):
                        emit_front(j)
                        for _ in range(2):
                            if pending:
                                oct2, st2 = pending.pop(0)
                                emit_stat_chunk(oct2, st2)
                    while pending:
                        oct2, st2 = pending.pop(0)
                        emit_stat_chunk(oct2, st2)
                    emit_mrep(1)
                    for j in range(n_t):
                        emit_back(j)
                else:
                    for j in range(n_t + 3):
                        if j < n_t:
                            emit_front(j)
                        if j > 2:
                            emit_back(j - 3)

            # group-0 stats, then main g0 with g1 stats interleaved
            for oct_ in range(4):
                for st in range(2 * (oct_ + 1)):
                    emit_stat_chunk(oct_, st)
            emit_mrep(0)
            pending = [(oct_, st) for oct_ in range(4, 8)
                       for st in range(2 * (oct_ + 1))]
            emit_main_group(0, pending)
            emit_main_group(1, [])

    nc.compile()
    return nc


def _host_inputs(q, k, v, w_q, w_k, w_v, w_o):
    """Build per-core input maps (host-side sharding + transposes)."""
    f16 = np.float16

    ident = np.eye(128, dtype=np.float32)
    negones = np.full((1, 128), -16.0, dtype=f16)
    ones_col = np.ones((128, 1), dtype=f16)

    # weights pre-arranged [128, 8*DK]: w_sb[p, di*DK+dk] = w[128*di+p, dk]
    def warr(w, dtype):
        return np.ascontiguousarray(
            w.reshape(8, 128, DK).transpose(1, 0, 2).reshape(128, 8 * DK)
        ).astype(dtype)
    wq_a = warr(w_q, np.float32)
    wk_a = warr(w_k, np.float32)
    wv_a = warr(w_v, f16)

    kidx = np.arange(128)[:, None]
    t_f = np.arange(1024)[None, :] // 128
    r_f = np.arange(1024)[None, :] % 128
    ridx = np.arange(128)[:, None]
    kk_f = np.arange(1024)[None, :]

    in_maps = []
    for c in range(N_CORES):
        blocks = [8 * o + (c if o % 2 == 0 else 7 - c) for o in range(NOCT)]
        q_rows = np.concatenate([q[128 * gb:128 * (gb + 1)] for gb in blocks])
        qT = np.ascontiguousarray(q_rows.T.reshape(8, 128, RPC))
        k_sh = k[KPC * c:KPC * (c + 1)]
        kT = np.ascontiguousarray(k_sh.T.reshape(8, 128, KPC))
        v_sh = v[KPC * c:KPC * (c + 1)]
        vT = np.ascontiguousarray(v_sh.T.reshape(8, 128, KPC)).astype(f16)
        maskT = np.empty((2, 128, 1024), np.float32)
        mask2 = np.empty((2, 128, 1024), np.float32)
        for p, pos in enumerate((c, 7 - c)):
            maskT[p] = np.where(128 * t_f + kidx <= 128 * pos + r_f, 0.0, NEG_BIG)
            mask2[p] = np.where(kk_f <= 128 * pos + ridx, 0.0, NEG_BIG)
        in_maps.append({
            "qT": qT, "kT": kT, "vT": vT,
            "wq": wq_a, "wk": wk_a, "wv": wv_a, "wo": w_o,
            "maskT": maskT, "mask2": mask2, "ident": ident,
            "negones": negones, "ones_col": ones_col,
        })
    return in_maps


def kernel(q, k, v, w_q, w_k, w_v, w_o):
    from concourse.bass_utils import run_bass_kernel_spmd

    q = np.asarray(q, dtype=np.float32)
    k = np.asarray(k, dtype=np.float32)
    v = np.asarray(v, dtype=np.float32)
    w_q = np.asarray(w_q, dtype=np.float32)
    w_k = np.asarray(w_k, dtype=np.float32)
    w_v = np.asarray(w_v, dtype=np.float32)
    w_o = np.asarray(w_o, dtype=np.float32)

    if "nc" not in _CACHE:
        _CACHE["nc"] = _build()
    nc = _CACHE["nc"]

    in_maps = _host_inputs(q, k, v, w_q, w_k, w_v, w_o)
    res = run_bass_kernel_spmd(nc, in_maps, list(range(N_CORES)))

    out = np.empty((S, D), dtype=np.float32)
    for c in range(N_CORES):
        o_sh = res.results[c]["o_sh"]
        for o in range(NOCT):
            gb = 8 * o + (c if o % 2 == 0 else 7 - c)
            out[128 * gb:128 * (gb + 1)] = o_sh[128 * o:128 * (o + 1)]
    return out
